# revision 2
# baseline (speedup 1.0000x reference)
"""MoE FFN (top-2 routing, 8 experts) on 8 Trainium2 NeuronCores.

Strategy (expert-pair x tensor-parallel hybrid):
  - Host computes router logits / top-2 / softmax (tiny: T x E) and
    gathers each expert's tokens.
  - Experts are paired (4 smallest loads with 4 largest) so each pair's
    token batch is ~T*K/4.  Each pair is served by TWO cores: both cores
    see all the pair's tokens, but each core holds only HALF of the F
    (FFN-intermediate) dimension of the pair's W1/W2.  F is the
    contraction dim of the second matmul, so each core emits a partial
    [H, C] output; the host sums the two partials (b2 is added on the
    even core only).  Per-core capacity is CA+CB ~ 2084 token-columns at
    half-F work == ~1042 token-equivalents, vs 1152 for plain expert
    parallelism: near-perfect balance across all 8 cores.
  - Core math (per chunk of <=512 token columns, fully transposed so
    weights are the stationary matmul operand):
        hT = GELU_tanh(W1s^T @ x + b1s)        [Fh, w]
        yT = wt * (W2s^T @ hT + b2s)           [H,  w]
    Matmul operands are fp16 (1 elem/cell/cycle on the PE, same as
    bf16); accumulation is fp32 in PSUM; bias/GELU/combine in fp32.
  - Chunks within a capacity slot are equal-width (<=512) to avoid
    narrow-matmul instruction-overhead floors.
  - Host scatter-adds each pair's summed partials back into [T, H].

Self-contained: hardcodes the problem shapes (H=768, F=3072, E=8, K=2).
"""

import os
import time

import numpy as np

H = 768
F = 3072
FH = F // 2          # per-core F slice
E = 8
K = 2
N_CORES = 8
P = 128
CHUNK = 512          # max token-chunk width (fp32 PSUM bank = 512 elems)

PRECISION = os.environ.get("MOE_PRECISION", "fp16")  # "fp16" | "bf16" | "fp32"
N_WARM = int(os.environ.get("MOE_WARMUP", "18"))


def _chunk_plan(CA, CB):
    """Equal-width chunks (<=CHUNK) per capacity slot; returns
    [(col0, width, slot), ...]."""
    plan = []
    off = 0
    for s, C in ((0, CA), (1, CB)):
        k = max(1, -(-C // CHUNK))
        base, rem = divmod(C, k)
        for i in range(k):
            w = base + (1 if i < rem else 0)
            plan.append((off, w, s))
            off += w
    return plan


# ---------------------------------------------------------------------------
# Bass/Tile device kernel
# ---------------------------------------------------------------------------

def _build_bass(CA, CB, precision=None):
    """Build + compile the per-core Bass program for slot capacities CA/CB."""
    from contextlib import ExitStack

    import concourse.bass as bass  # noqa: F401
    import concourse.tile as tile
    from concourse import bacc, mybir
    from concourse._compat import with_exitstack

    precision = precision or PRECISION
    CT = CA + CB
    FM = FH // P         # 12: F-half tiles (phase-A psum rows / phase-B k)
    HK = H // P          # 6: contraction tiles for x@W1
    HN = H // P          # 6: output row tiles of yT
    f32 = mybir.dt.float32
    mdt = {"bf16": mybir.dt.bfloat16, "fp16": mybir.dt.float16,
           "fp32": f32}[precision]

    chunks = _chunk_plan(CA, CB)

    nc = bacc.Bacc("TRN2", target_bir_lowering=False, debug=False,
                   num_devices=N_CORES)
    # xgT: slot-A token columns [0,CA), slot-B [CA,CA+CB)
    xgT = nc.dram_tensor("xgt", [H, CT], mdt, kind="ExternalInput").ap()
    # w1: [H, 2*FH] = [exp-A F-half | exp-B F-half]
    w1 = nc.dram_tensor("w1", [H, 2 * FH], mdt, kind="ExternalInput").ap()
    # w2: [2*FH, H] = [exp-A F-half rows ; exp-B F-half rows]
    w2 = nc.dram_tensor("w2", [2 * FH, H], mdt, kind="ExternalInput").ap()
    # packed fp32 constants along free dim:
    # [b1A(FM) | b1B(FM) | b2A(HN) | b2B(HN) | wt(CT)]
    NCONST = 2 * FM + 2 * HN
    cpk = nc.dram_tensor("cpk", [P, NCONST + CT], f32,
                         kind="ExternalInput").ap()
    y = nc.dram_tensor("y", [H, CT], f32, kind="ExternalOutput").ap()

    gelu = mybir.ActivationFunctionType.Gelu_apprx_tanh
    ident = mybir.ActivationFunctionType.Identity

    NW1 = 4              # DMA pieces per W1 slot-half (FQ cols each)
    FQ = FH // NW1       # 384
    FQT = FQ // P        # 3 fm-tiles per piece

    @with_exitstack
    def body(ctx: ExitStack, tc: tile.TileContext):
        const = ctx.enter_context(tc.tile_pool(name="const", bufs=1))
        w1p = ctx.enter_context(tc.tile_pool(name="w1p", bufs=1))
        w2p = ctx.enter_context(tc.tile_pool(name="w2p", bufs=1))
        xp = ctx.enter_context(tc.tile_pool(name="xp", bufs=1))
        hp = ctx.enter_context(tc.tile_pool(name="hp", bufs=1))
        yp = ctx.enter_context(tc.tile_pool(name="yp", bufs=3))
        psAp = ctx.enter_context(tc.tile_pool(name="psA", bufs=2, space="PSUM"))
        psBp = ctx.enter_context(tc.tile_pool(name="psB", bufs=6, space="PSUM"))

        # Pre-warm the PE's HAM clock gate during the DMA-bound startup:
        # dummy matmuls on a memset tile (no load dependency) lift the PE
        # clock 1.2 -> 2.4 GHz before the real data lands.
        wtile = xp.tile([P, CHUNK], mdt, tag="warm", name="warm")
        nc.vector.memset(wtile[:], 0.0)
        wps = psBp.tile([P, CHUNK], f32, tag="psB", name="warmps")
        for i in range(N_WARM):
            nc.tensor.matmul(wps[:], lhsT=wtile[:, 0:P], rhs=wtile[:],
                             start=(i == 0), stop=(i == N_WARM - 1))

        # ---- input DMAs, in order of first use; both HWDGE rings pull
        # from the same ~360 GB/s per-core HBM pipe, so what matters is
        # issue order per ring (FIFO) and splitting bulk across rings.
        c00, w0, _ = chunks[0]
        xg = []
        t = xp.tile([P, HK, w0], mdt, tag="xg0", name="xg0")
        nc.sync.dma_start(
            t[:], xgT[:, 0:w0].rearrange("(k p) c -> p k c", p=P))
        xg.append(t)

        # W1 slot-A piece 0 gates the first matmul group: scalar ring.
        w1q = [None] * (2 * NW1)

        def load_w1(s, g, eng):
            tq = w1p.tile([P, HK, FQ], mdt, tag=f"w1q{s}_{g}",
                          name=f"w1q{s}_{g}")
            src = w1[:, s * FH + g * FQ:s * FH + (g + 1) * FQ].rearrange(
                "(k p) f -> p k f", p=P)
            eng.dma_start(tq[:], src)
            w1q[s * NW1 + g] = tq

        load_w1(0, 0, nc.scalar)
        # constants (b1 needed by first activation)
        cps = const.tile([P, NCONST + CT], f32, name="cps")
        nc.scalar.dma_start(cps[:], cpk[:])
        b1s = [cps[:, 0:FM], cps[:, FM:2 * FM]]
        b2s = [cps[:, 2 * FM:2 * FM + HN], cps[:, 2 * FM + HN:NCONST]]
        wtbs = cps[:, NCONST:]

        load_w1(0, 1, nc.sync)
        load_w1(0, 2, nc.scalar)
        load_w1(0, 3, nc.sync)

        # W2 per slot: 12 row-tiles; split each slot-half across rings.
        W2G = FM // 2  # 6 row-tiles per grouped DMA
        w2g = [None] * 4

        def load_w2(s, g, eng):
            tg = w2p.tile([P, W2G, H], mdt, tag=f"w2g{s}_{g}",
                          name=f"w2g{s}_{g}")
            r0 = s * FH + g * W2G * P
            src = w2[r0:r0 + W2G * P, :].rearrange("(k p) f -> p k f", p=P)
            eng.dma_start(tg[:], src)
            w2g[s * 2 + g] = tg

        load_w2(0, 0, nc.scalar)
        load_w2(0, 1, nc.sync)

        # remaining x chunks, then slot-B weights
        for i, (c0, w, _s) in enumerate(chunks[1:], start=1):
            t = xp.tile([P, HK, w], mdt, tag=f"xg{i}", name=f"xg{i}")
            (nc.sync if i % 2 else nc.scalar).dma_start(
                t[:], xgT[:, c0:c0 + w].rearrange("(k p) c -> p k c", p=P))
            xg.append(t)
        load_w1(1, 0, nc.scalar)
        load_w1(1, 1, nc.sync)
        load_w1(1, 2, nc.scalar)
        load_w1(1, 3, nc.sync)
        load_w2(1, 0, nc.scalar)
        load_w2(1, 1, nc.sync)

        def w1_tile(s, hk, fm):
            return w1q[s * NW1 + fm // FQT][
                :, hk, (fm % FQT) * P:(fm % FQT + 1) * P]

        def w2_tile(s, fk):
            return w2g[s * 2 + fk // W2G][:, fk % W2G, :]

        for ci, (c0, w, s) in enumerate(chunks):
            # ---- phase A: hT[f, c] = gelu((x@W1s)[c, f] + b1s[f]) ----
            hts = [None] * FM
            for fm in range(FM):
                ps = psAp.tile([P, CHUNK], f32, tag="psA", name="psA")
                for hk in range(HK):
                    nc.tensor.matmul(
                        ps[:, :w],
                        lhsT=w1_tile(s, hk, fm),
                        rhs=xg[ci][:, hk, :w],
                        start=(hk == 0), stop=(hk == HK - 1),
                    )
                ht = hp.tile([P, CHUNK], mdt, tag=f"hts{fm}",
                             name=f"hts{fm}")
                nc.scalar.activation(ht[:, :w], ps[:, :w], gelu,
                                     bias=b1s[s][:, fm:fm + 1])
                hts[fm] = ht

            # ---- phase B: yT[h, c] = sum_f W2s[f, h] * hT[f, c] ----
            for hn in range(HN):
                ps = psBp.tile([P, CHUNK], f32, tag="psB", name="psB")
                for fk in range(FM):
                    nc.tensor.matmul(
                        ps[:, :w],
                        lhsT=w2_tile(s, fk)[:, hn * P:(hn + 1) * P],
                        rhs=hts[fk][:, :w],
                        start=(fk == 0), stop=(fk == FM - 1),
                    )
                # ---- epilogue: (+b2), (*wt), store ----
                ot = yp.tile([P, CHUNK], f32, tag="yout", name="yout")
                nc.scalar.activation(ot[:, :w], ps[:, :w], ident,
                                     bias=b2s[s][:, hn:hn + 1])
                ot2 = yp.tile([P, CHUNK], f32, tag="yout2", name="yout2")
                nc.vector.tensor_mul(ot2[:, :w], ot[:, :w],
                                     wtbs[:, c0:c0 + w])
                nc.sync.dma_start(y[hn * P:(hn + 1) * P, c0:c0 + w],
                                  ot2[:, :w])

    with tile.TileContext(nc) as tc:
        body(tc)
    nc.compile()
    return nc


# ---------------------------------------------------------------------------
# Host-side routing + dispatch
# ---------------------------------------------------------------------------

def _route(xf, gate_w):
    """Top-2 router in float64 for a numerically robust top-k set.

    Returns per-expert (token_idx, weight) lists.
    """
    logits = xf.astype(np.float64) @ gate_w.astype(np.float64)  # [T, E]
    top_idx = np.argpartition(logits, E - K, axis=1)[:, E - K:]  # [T, K]
    top_val = np.take_along_axis(logits, top_idx, axis=1)
    m = top_val.max(axis=1, keepdims=True)
    ex = np.exp(top_val - m)
    wts = ex / ex.sum(axis=1, keepdims=True)  # [T, K] float64

    toks, ws = [], []
    for e in range(E):
        mask = top_idx == e  # [T, K]
        rows = np.nonzero(mask.any(axis=1))[0]
        toks.append(rows)
        ws.append(wts[mask].astype(np.float32))
    return toks, ws


def _np_mdt():
    import ml_dtypes
    return {"bf16": ml_dtypes.bfloat16, "fp16": np.float16,
            "fp32": np.float32}[PRECISION]


def _make_in_maps(xf, gate_w, W1, b1, W2, b2):
    toks, ws = _route(xf, gate_w)
    n = [len(t) for t in toks]
    order = list(np.argsort(n))
    pairs = [(order[i], order[E - 1 - i]) for i in range(E // 2)]
    CA = max(1, max(n[a] for a, _ in pairs))
    CB = max(1, max(n[b] for _, b in pairs))
    CT = CA + CB
    mdt = _np_mdt()

    W1a = np.asarray(W1, np.float32)
    b1a = np.asarray(b1, np.float32)
    W2a = np.asarray(W2, np.float32)
    b2a = np.asarray(b2, np.float32)
    in_maps = []
    for p_i, (a, b) in enumerate(pairs):
        xgT = np.zeros((H, CT), mdt)
        xgT[:, :n[a]] = xf[toks[a]].T.astype(mdt)
        xgT[:, CA:CA + n[b]] = xf[toks[b]].T.astype(mdt)
        wtb = np.zeros((P, CT), np.float32)
        wtb[:, :n[a]] = ws[a][None, :]
        wtb[:, CA:CA + n[b]] = ws[b][None, :]
        for half in range(2):
            fc = slice(half * FH, (half + 1) * FH)
            w1pk = np.concatenate(
                [W1a[a][:, fc], W1a[b][:, fc]], axis=1).astype(mdt)
            w2pk = np.concatenate(
                [W2a[a][fc, :], W2a[b][fc, :]], axis=0).astype(mdt)
            # b2 only on the even core (partials are summed on host)
            b2c = (b2a if half == 0 else np.zeros_like(b2a))
            cpk = np.concatenate([
                b1a[a][fc].reshape(FH // P, P).T,
                b1a[b][fc].reshape(FH // P, P).T,
                b2c[a].reshape(H // P, P).T,
                b2c[b].reshape(H // P, P).T,
                wtb,
            ], axis=1)
            in_maps.append({
                "xgt": np.ascontiguousarray(xgT),
                "w1": np.ascontiguousarray(w1pk),
                "w2": np.ascontiguousarray(w2pk),
                "cpk": np.ascontiguousarray(cpk),
            })
    return in_maps, toks, pairs, n, CA, CB


def _run(inputs, trace=False):
    global PRECISION
    from concourse.bass_utils import run_bass_kernel_spmd

    x, gate_w, W1, b1, W2, b2 = (inputs[k] for k in
                                 ("x", "gate_w", "W1", "b1", "W2", "b2"))
    x = np.asarray(x)
    Bb, S, Hd = x.shape
    assert Hd == H
    T = Bb * S
    xf = np.ascontiguousarray(x.reshape(T, Hd), dtype=np.float32)
    gate_w = np.asarray(gate_w, np.float32)

    # fp16 matmul operands need moderate dynamic range; fall back to
    # bf16 (full fp32 exponent range) if the data is far outside the
    # expected unit-scale regime.
    if PRECISION == "fp16":
        amax = max(float(np.abs(np.asarray(t)).max())
                   for t in (xf, W1, W2))
        if not np.isfinite(amax) or amax > 1e3:
            PRECISION = "bf16"

    in_maps, toks, pairs, n, CA, CB = _make_in_maps(
        xf, gate_w, W1, b1, W2, b2)
    nc = _build_bass(CA, CB)

    kwargs = {}
    if trace:
        kwargs = dict(trace=True, trace_cores=list(range(N_CORES)))
    try:
        res = run_bass_kernel_spmd(nc, in_maps, core_ids=list(range(N_CORES)),
                                   **kwargs)
    except Exception:
        # One retry for transient device faults.
        time.sleep(5)
        res = run_bass_kernel_spmd(nc, in_maps, core_ids=list(range(N_CORES)),
                                   **kwargs)
    out = np.zeros((T, H), np.float32)
    for p_i, (a, b) in enumerate(pairs):
        ysum = res.results[2 * p_i]["y"] + res.results[2 * p_i + 1]["y"]
        out[toks[a]] += ysum[:, :n[a]].T
        out[toks[b]] += ysum[:, CA:CA + n[b]].T
    return out.reshape(Bb, S, Hd), res


def kernel(x, gate_w, W1, b1, W2, b2):
    out, _ = _run({"x": x, "gate_w": gate_w, "W1": W1, "b1": b1,
                   "W2": W2, "b2": b2})
    return out.astype(np.asarray(x).dtype, copy=False)


# Exposed for test.py: run with profiling, return (output, BassKernelResults)
def kernel_profiled(x, gate_w, W1, b1, W2, b2):
    return _run({"x": x, "gate_w": gate_w, "W1": W1, "b1": b1,
                 "W2": W2, "b2": b2}, trace=True)
